# revision 10
# baseline (speedup 1.0000x reference)
"""Expert-parallel top-2 MoE kernel for 8 Trainium2 NeuronCores.

Strategy (expert-parallel, sparse dispatch, per the sharding hint):
  - Router sharded over cores: core c computes fp32 logits for its 512-token
    shard on the TensorEngine (Wg stationary) in [E, 512] layout, AllGathers
    the contiguous [E,512] blocks, and transposes to token-partition layout
    on the TensorEngine; softmax/top-2 on-device (top-2 indicator built with
    5 full-width vector ops via masked second-max).
  - Core c owns expert c. Slot positions come from matmul-based exclusive
    cumsums (single full-width matmuls). Dispatch builds the slot->token map
    by INDIRECT-DMA SCATTER of 4-byte token ids into a [C] DRAM vector
    (tokens not routed here get an out-of-bounds slot id and are silently
    skipped via bounds_check), loads it back contiguously, transposes it to
    slot-tile layout on the TensorEngine, then indirect-gathers just the C
    routed token rows from DRAM and transposes them into xgT for the FFN.
  - Two-layer FFN in bf16 over the routed tokens (capacity padded to a
    multiple of 384), in uneven groups {384,384,256,128}; each group's
    unscaled outputs are AllGathered right after they finish and combined
    (indirect gather + gate-weighted fp32 accumulation) while the next
    group's FFN runs. Only the small last chunk's AllGather is exposed.
  - Each core returns its own 512-token shard; host concatenates.

Numerics: router fp32 (top-2 selection fidelity), FFN bf16 with fp32
accumulation in PSUM, combine in fp32.
"""

import os
import sys

import numpy as np

for _p in ("/opt/trn_rl_repo",):
    if _p not in sys.path:
        sys.path.append(_p)

import ml_dtypes

import concourse.bass as bass
import concourse.mybir as mybir
import concourse.tile as tile
from concourse import bacc
from concourse.bass import IndirectOffsetOnAxis
from concourse.masks import make_identity

# Problem shapes (fixed per spec)
B, S, D, E = 2, 2048, 1024, 8
T = B * S          # 4096 tokens
F = 4 * D          # 4096 ffn dim
P = 128            # partitions
NT = T // P        # 32 token tiles
KD = D // P        # 8 contraction tiles over D
NF = F // P        # 32 f tiles
TOK_PER_CORE = T // E   # 512
OWN_TILES = TOK_PER_CORE // P  # 4
N_CORES = E
CHK = 3 * P                  # 384: capacity rounding unit
BIGP = 100000.0              # OOB slot id for tokens not routed here

f32 = mybir.dt.float32
bf16 = mybir.dt.bfloat16
i32 = mybir.dt.int32
u32 = mybir.dt.uint32

_cache = {}


def _group_sizes(ST):
    """Slot-tile counts per FFN group; last groups shrink so the exposed
    final AllGather is small. ST=9 -> [3, 3, 2, 1]."""
    assert ST % 3 == 0 and ST >= 3
    n3 = ST // 3
    if n3 == 1:
        return [2, 1]
    return [3] * (n3 - 1) + [2, 1]


def build_module(C: int, debug_out: bool = False):
    """Build the SPMD Bass module for capacity C (multiple of 384)."""
    assert C % CHK == 0
    ST = C // P  # slot tiles per expert

    nc = bacc.Bacc("TRN2", target_bir_lowering=False, debug=False,
                   num_devices=N_CORES)

    # ---- I/O ----
    xTs = nc.dram_tensor("xTs", [D, TOK_PER_CORE], f32,
                         kind="ExternalInput").ap()
    xbf = nc.dram_tensor("xbf", [T, D], bf16, kind="ExternalInput").ap()
    w1d = nc.dram_tensor("w1d", [D, F], bf16, kind="ExternalInput").ap()
    w2d = nc.dram_tensor("w2d", [F, D], bf16, kind="ExternalInput").ap()
    wgd = nc.dram_tensor("wgd", [D, E], f32, kind="ExternalInput").ap()
    bgb = nc.dram_tensor("bgb", [P, NT * E], f32, kind="ExternalInput").ap()
    b1pm = nc.dram_tensor("b1pm", [P, NF], f32, kind="ExternalInput").ap()
    b2r = nc.dram_tensor("b2r", [1, D], f32, kind="ExternalInput").ap()
    sel256 = nc.dram_tensor("sel256", [P, NT * E], f32,
                            kind="ExternalInput").ap()
    l128d = nc.dram_tensor("l128d", [P, P], f32, kind="ExternalInput").ap()
    ownmd = nc.dram_tensor("ownmd", [P, OWN_TILES * NT], f32,
                           kind="ExternalInput").ap()
    out = nc.dram_tensor("out", [TOK_PER_CORE, D], f32,
                         kind="ExternalOutput").ap()
    dbg = None
    if debug_out:
        GS = _group_sizes(ST)
        dbg = {
            "dbg_l": nc.dram_tensor("dbg_l", [P, NT * E], f32,
                                    kind="ExternalOutput").ap(),
            "dbg_pos": nc.dram_tensor("dbg_pos", [P, NT * E], f32,
                                      kind="ExternalOutput").ap(),
            "dbg_ind": nc.dram_tensor("dbg_ind", [P, NT * E], f32,
                                      kind="ExternalOutput").ap(),
            "dbg_posm": nc.dram_tensor("dbg_posm", [P, NT], i32,
                                       kind="ExternalOutput").ap(),
            "dbg_idx": nc.dram_tensor("dbg_idx", [P, ST], i32,
                                      kind="ExternalOutput").ap(),
            "dbg_xgT": nc.dram_tensor("dbg_xgT", [P, C], bf16,
                                      kind="ExternalOutput").ap(),
            "dbg_red": nc.dram_tensor(
                "dbg_red", [P, OWN_TILES * 4 * len(GS)], f32,
                kind="ExternalOutput").ap(),
            "dbg_y": [nc.dram_tensor(f"dbg_y{g}", [N_CORES * GS[g] * P, D],
                                     bf16, kind="ExternalOutput").ap()
                      for g in range(len(GS))],
        }

    with tile.TileContext(nc) as tc:
        _emit(tc, C, ST, xTs, xbf, w1d, w2d, wgd, bgb, b1pm, b2r, sel256,
              l128d, ownmd, out, dbg)

    nc.compile()
    return nc


def _emit(tc, C, ST, xTs, xbf, w1d, w2d, wgd, bgb, b1pm, b2r, sel256,
          l128d, ownmd, out, dbg=None):
    nc = tc.nc
    NE = NT * E  # 256
    GS = _group_sizes(ST)        # slot tiles per group
    NG = len(GS)
    NSEL = 2 * NG
    GSTART = [sum(GS[:g]) * P for g in range(NG)]   # slot offset per group
    GROWS = [GS[g] * P for g in range(NG)]          # rows per group chunk

    # ---------------- persistent pools ----------------
    persist = tc.alloc_tile_pool(name="persist", bufs=1)
    dram = tc.alloc_tile_pool(name="dram", bufs=1, space="DRAM")

    # tiny warmup AllGather: absorbs first-collective setup cost and aligns
    # the cores before the logits AllGather on the critical path
    wup_in = dram.tile([E, 4], f32, name="wup_in")
    wup_out = dram.tile([N_CORES * E, 4], f32, addr_space="Shared",
                        name="wup_out")
    nc.gpsimd.collective_compute(
        "AllGather", mybir.AluOpType.bypass,
        replica_groups=[list(range(N_CORES))],
        ins=[wup_in[:].opt()], outs=[wup_out[:].opt()],
    )

    # router inputs first (split across DMA queues so the router starts fast)
    wg_sb = persist.tile([P, KD, E], f32, name="wg_sb")
    nc.sync.dma_start(wg_sb[:], wgd.rearrange("(k p) e -> p k e", p=P))
    xs = persist.tile([P, KD, TOK_PER_CORE], f32, name="xs")
    xsv = xTs.rearrange("(k p) t -> p k t", p=P)
    for k in range(KD):
        nc.sync.dma_start(xs[:, k, :], xsv[:, k, :])
    bg_sb = persist.tile([P, NE], f32, name="bg_sb")
    nc.sync.dma_start(bg_sb[:], bgb[:])
    sel_sb = persist.tile([P, NE], f32, name="sel_sb")
    nc.sync.dma_start(sel_sb[:], sel256[:])
    l128_sb = persist.tile([P, P], f32, name="l128_sb")
    nc.sync.dma_start(l128_sb[:], l128d[:])
    ownm_sb = persist.tile([P, OWN_TILES * NT], f32, name="ownm_sb")
    nc.sync.dma_start(ownm_sb[:], ownmd[:])
    b1_sb = persist.tile([P, NF], f32, name="b1_sb")
    nc.sync.dma_start(b1_sb[:], b1pm[:])
    b2_sb = persist.tile([1, D], f32, name="b2_sb")
    nc.sync.dma_start(b2_sb[:], b2r[:])
    ident = persist.tile([P, P], bf16, name="ident")
    make_identity(nc, ident[:])
    identf = persist.tile([P, P], f32, name="identf")
    make_identity(nc, identf[:])
    ones_col = persist.tile([P, 1], f32, name="ones_col")
    nc.vector.memset(ones_col[:], 1.0)
    ones_row = persist.tile([1, P], f32, name="ones_row")
    nc.vector.memset(ones_row[:], 1.0)
    # token ids (f32) for the id-scatter; zero image for the idx vector
    tokf = persist.tile([P, NT], f32, name="tokf")
    toki = persist.tile([P, NT], i32, name="toki")
    nc.gpsimd.iota(toki[:], pattern=[[P, NT]], base=0, channel_multiplier=1)
    nc.vector.tensor_copy(tokf[:], toki[:])
    zrow = persist.tile([1, C], f32, name="zrow")
    nc.vector.memset(zrow[:], 0.0)

    idxr_dram = dram.tile([C, 1], f32, name="idxr_dram")
    nc.sync.dma_start(idxr_dram[:].rearrange("c o -> o c"), zrow[:])

    w1_sb = [persist.tile([P, F], bf16, name=f"w1_sb{k}") for k in range(KD)]
    for k in range(KD):
        nc.sync.dma_start(w1_sb[k][:], w1d[k * P:(k + 1) * P, :])

    # router / dispatch state kept for the combine phase
    exp_all = persist.tile([P, NE], f32, name="exp_all")    # exp(logits)
    m8_all = persist.tile([P, NE], f32, name="m8_all")      # per-tile top8
    r_all = persist.tile([P, NT], f32, name="r_all")        # 1/sum(exp)
    pos_all = persist.tile([P, NE], f32, name="pos_all")    # excl cumsum
    ind_all = persist.tile([P, NE], f32, name="ind_all")    # top2 indicator
    ei_all = persist.tile([P, NE], u32, name="ei_all")      # top8 indices
    posm_i = persist.tile([P, NT], i32, name="posm_i")      # scatter slots
    red_sb = persist.tile([P, OWN_TILES, 2 * NSEL], f32, name="red_sb")
    redi_sb = persist.tile([P, OWN_TILES, NSEL], i32, name="redi_sb")
    ot = [persist.tile([P, D], f32, name=f"ot{j}") for j in range(OWN_TILES)]

    xgT = [persist.tile([P, C], bf16, name=f"xgT{d}") for d in range(KD)]

    l_dram = dram.tile([E, TOK_PER_CORE], f32, name="l_dram")
    lg_dram = dram.tile([N_CORES * E, TOK_PER_CORE], f32, addr_space="Shared",
                        name="lg_dram")
    y_dram = [dram.tile([GROWS[g], D], bf16, name=f"y_dram{g}")
              for g in range(NG)]
    y_all = [dram.tile([N_CORES * GROWS[g], D], bf16, addr_space="Shared",
                       name=f"y_all{g}") for g in range(NG)]

    # ---------------- router (sharded + AllGather) ----------------
    with tc.tile_pool(name="router_sb", bufs=1, named_scope="router") as rpool, \
         tc.tile_pool(name="router_ps", bufs=1, space="PSUM") as rps:
        lT = rps.tile([E, TOK_PER_CORE], f32, name="lT")
        for k in range(KD):
            nc.tensor.matmul(lT[:], lhsT=wg_sb[:, k, :], rhs=xs[:, k, :],
                             start=(k == 0), stop=(k == KD - 1))
        lt_sb = rpool.tile([E, TOK_PER_CORE], f32, name="lt_sb")
        nc.vector.tensor_copy(lt_sb[:], lT[:])
        nc.sync.dma_start(l_dram[:], lt_sb[:])
        nc.gpsimd.collective_compute(
            "AllGather", mybir.AluOpType.bypass,
            replica_groups=[list(range(N_CORES))],
            ins=[l_dram[:].opt()], outs=[lg_dram[:].opt()],
        )
        # stream gathered logits per core (sources at partition 0), then
        # transpose each 128-token tile to token-partition [P, (t e)] layout
        l_all = rpool.tile([P, NE], f32, name="l_all")
        QT = TOK_PER_CORE // P  # 4 token tiles per core shard
        for c in range(N_CORES):
            lgc = rpool.tile([E, TOK_PER_CORE], f32, tag="lgc", bufs=2,
                             name="lgc")
            nc.sync.dma_start(lgc[:], lg_dram[c * E:(c + 1) * E, :])
            pt = rps.tile([P, QT, E], f32, tag="pt", bufs=2, name="pt")
            for q in range(QT):
                nc.tensor.transpose(
                    pt[:, q, :], lgc[:, q * P:(q + 1) * P], identf[:E, :E])
            nc.vector.tensor_copy(
                l_all[:, c * QT * E:(c + 1) * QT * E], pt[:])
        nc.vector.tensor_add(l_all[:], l_all[:], bg_sb[:])
        if dbg is not None:
            nc.sync.dma_start(dbg["dbg_l"][:], l_all[:])
        nc.scalar.activation(exp_all[:], l_all[:],
                             mybir.ActivationFunctionType.Exp)
        # top-2 indicator via masked second-max (5 full-width ops)
        exp3 = exp_all[:].rearrange("p (t e) -> p t e", e=E)
        mx = rpool.tile([P, NT], f32, name="mx")
        nc.vector.reduce_max(mx[:], exp3, axis=mybir.AxisListType.X)
        eqm = rpool.tile([P, NE], f32, name="eqm")
        nc.vector.tensor_tensor(
            out=eqm[:].rearrange("p (t e) -> p t e", e=E), in0=exp3,
            in1=mx[:, :, None].to_broadcast([P, NT, E]),
            op=mybir.AluOpType.is_ge)
        nm = rpool.tile([P, NE], f32, name="nm")
        nc.vector.scalar_tensor_tensor(
            out=nm[:], in0=eqm[:], scalar=-BIGP, in1=exp_all[:],
            op0=mybir.AluOpType.mult, op1=mybir.AluOpType.add)
        m2 = rpool.tile([P, NT], f32, name="m2")
        nc.vector.reduce_max(m2[:], nm[:].rearrange("p (t e) -> p t e", e=E),
                             axis=mybir.AxisListType.X)
        nc.vector.tensor_tensor(
            out=ind_all[:].rearrange("p (t e) -> p t e", e=E), in0=exp3,
            in1=m2[:, :, None].to_broadcast([P, NT, E]),
            op=mybir.AluOpType.is_ge)

    # ---------------- dispatch: cumsum positions + id scatter ----------------
    with tc.tile_pool(name="disp_sb", bufs=1, named_scope="dispatch") as dpool, \
         tc.tile_pool(name="disp_ps", bufs=1, space="PSUM") as dps:
        # per-(tile,expert) totals in one matmul
        ptot = dps.tile([1, NE], f32, name="ptot")
        nc.tensor.matmul(ptot[:], lhsT=ones_col[:], rhs=ind_all[:],
                         start=True, stop=True)
        tot_flat = dpool.tile([1, NE], f32, name="tot_flat")
        nc.vector.tensor_copy(tot_flat[:], ptot[:])
        # reshape [1, NT*E] -> [NT, E] via sbuf-to-sbuf DMA
        tot32 = dpool.tile([NT, E], f32, name="tot32")
        nc.sync.dma_start(tot32[:], tot_flat[:])
        # exclusive cumsum over tiles: strict-lower matmul
        pofs = dps.tile([NT, E], f32, name="pofs")
        nc.tensor.matmul(pofs[:], lhsT=l128_sb[:NT, :NT], rhs=tot32[:],
                         start=True, stop=True)
        ofs32 = dpool.tile([NT, E], f32, name="ofs32")
        nc.vector.tensor_copy(ofs32[:], pofs[:])
        ofs_flat = dpool.tile([1, NE], f32, name="ofs_flat")
        nc.sync.dma_start(ofs_flat[:], ofs32[:])
        # positions: local excl cumsum + broadcast tile offset, one pass
        ppos = dps.tile([P, NE], f32, name="ppos")
        nc.tensor.matmul(ppos[:], lhsT=l128_sb[:], rhs=ind_all[:],
                         start=True, stop=False)
        nc.tensor.matmul(ppos[:], lhsT=ones_row[:], rhs=ofs_flat[:],
                         start=False, stop=True)
        nc.vector.tensor_copy(pos_all[:], ppos[:])

        # my expert's masked positions: ind ? pos : BIGP (skipped as OOB)
        tmp = dpool.tile([P, NE], f32, name="tmp")
        nc.vector.tensor_mul(tmp[:], pos_all[:], sel_sb[:])
        pos_e = dpool.tile([P, NT], f32, name="pos_e")
        nc.vector.reduce_sum(pos_e[:], tmp[:].rearrange(
            "p (t e) -> p t e", e=E), axis=mybir.AxisListType.X)
        nc.vector.tensor_mul(tmp[:], ind_all[:], sel_sb[:])
        ind_e = dpool.tile([P, NT], f32, name="ind_e")
        nc.vector.reduce_sum(ind_e[:], tmp[:].rearrange(
            "p (t e) -> p t e", e=E), axis=mybir.AxisListType.X)
        pos_m = dpool.tile([P, NT], f32, name="pos_m")
        nc.vector.tensor_scalar_add(pos_m[:], pos_e[:], -BIGP)
        nc.vector.tensor_mul(pos_m[:], pos_m[:], ind_e[:])
        nc.vector.tensor_scalar_add(pos_m[:], pos_m[:], BIGP)
        nc.vector.tensor_copy(posm_i[:], pos_m[:])
        if dbg is not None:
            nc.sync.dma_start(dbg["dbg_pos"][:], pos_all[:])
            nc.sync.dma_start(dbg["dbg_ind"][:], ind_all[:])
            nc.sync.dma_start(dbg["dbg_posm"][:], posm_i[:])

        # scatter token ids (4B each) into their slots
        for tt in range(NT):
            nc.gpsimd.indirect_dma_start(
                out=idxr_dram[:], out_offset=IndirectOffsetOnAxis(
                    ap=posm_i[:, tt:tt + 1], axis=0),
                in_=tokf[:, tt:tt + 1], in_offset=None,
                bounds_check=C - 1, oob_is_err=False,
            )

        # off the critical path: top-8 values/indices + softmax denom + the
        # combine selection stack (vector work that overlaps the scatter)
        for tt in range(NT):
            sl = slice(tt * E, (tt + 1) * E)
            nc.vector.max(out=m8_all[:, sl], in_=exp_all[:, sl])
            nc.vector.max_index(out=ei_all[:, sl], in_max=m8_all[:, sl],
                                in_values=exp_all[:, sl])
        s_all = dpool.tile([P, NT], f32, name="s_all")
        nc.vector.reduce_sum(s_all[:], exp_all[:].rearrange(
            "p (t e) -> p t e", e=E), axis=mybir.AxisListType.X)
        nc.vector.reciprocal(r_all[:], s_all[:])

        # selection stack: NSEL offset planes + NSEL weight planes [P, NT]
        e1f = dpool.tile([P, NT], f32, name="e1f")
        e2f = dpool.tile([P, NT], f32, name="e2f")
        ei3 = ei_all[:].rearrange("p (t e) -> p t e", e=E)
        nc.vector.tensor_copy(e1f[:], ei3[:, :, 0])
        nc.vector.tensor_copy(e2f[:], ei3[:, :, 1])
        ioz = dpool.tile([P, NE], i32, name="ioz")
        nc.gpsimd.iota(ioz[:].rearrange("p (t e) -> p t e", e=E),
                       pattern=[[0, NT], [1, E]], base=0, channel_multiplier=0)
        iof = dpool.tile([P, NE], f32, name="iof")
        nc.vector.tensor_copy(iof[:], ioz[:])
        m83 = m8_all[:].rearrange("p (t e) -> p t e", e=E)
        Ssel = dpool.tile([P, 2 * NSEL, NT], f32, name="Ssel")
        for q, ef in ((0, e1f), (1, e2f)):
            oh = dpool.tile([P, NE], f32, tag=f"oh{q}", name=f"oh{q}")
            nc.vector.tensor_tensor(
                out=oh[:].rearrange("p (t e) -> p t e", e=E),
                in0=iof[:].rearrange("p (t e) -> p t e", e=E),
                in1=ef[:, :, None].to_broadcast([P, NT, E]),
                op=mybir.AluOpType.is_equal)
            nc.vector.tensor_mul(oh[:], oh[:], pos_all[:])
            slot = dpool.tile([P, NT], f32, tag=f"slot{q}", name=f"slot{q}")
            nc.vector.reduce_sum(slot[:], oh[:].rearrange(
                "p (t e) -> p t e", e=E), axis=mybir.AxisListType.X)
            # chunk id g: count of group starts <= slot (starts beyond 0)
            gch = dpool.tile([P, NT], f32, tag=f"gch{q}", name=f"gch{q}")
            nc.vector.tensor_scalar(gch[:], slot[:], float(GSTART[1]), None,
                                    op0=mybir.AluOpType.is_ge)
            for gg in range(2, NG):
                t2 = dpool.tile([P, NT], f32, tag="t2", name="t2")
                nc.vector.tensor_scalar(t2[:], slot[:], float(GSTART[gg]),
                                        None, op0=mybir.AluOpType.is_ge)
                nc.vector.tensor_add(gch[:], gch[:], t2[:])
            gv = dpool.tile([P, NT], f32, tag=f"gv{q}", name=f"gv{q}")
            nc.vector.tensor_tensor(out=gv[:], in0=m83[:, :, q], in1=r_all[:],
                                    op=mybir.AluOpType.mult)
            for gg in range(NG):
                k = q * NG + gg
                # row in y_all[gg]: e*GROWS[gg] + slot - GSTART[gg]
                eq = dpool.tile([P, NT], f32, tag="eq", name="eq")
                nc.vector.tensor_scalar(eq[:], gch[:], float(gg), None,
                                        op0=mybir.AluOpType.is_equal)
                nc.vector.scalar_tensor_tensor(
                    out=Ssel[:, k, :], in0=ef[:], scalar=float(GROWS[gg]),
                    in1=slot[:], op0=mybir.AluOpType.mult,
                    op1=mybir.AluOpType.add)
                nc.vector.tensor_scalar_add(Ssel[:, k, :], Ssel[:, k, :],
                                            float(-GSTART[gg]))
                nc.vector.tensor_mul(Ssel[:, k, :], Ssel[:, k, :], eq[:])
                nc.vector.tensor_mul(Ssel[:, NSEL + k, :], eq[:], gv[:])
        for j in range(OWN_TILES):
            own = ownm_sb[:, j * NT:(j + 1) * NT]
            tmpS = dpool.tile([P, 2 * NSEL, NT], f32, tag="tmpS", bufs=2,
                              name="tmpS")
            nc.vector.tensor_tensor(
                out=tmpS[:], in0=Ssel[:],
                in1=own[:, None, :].to_broadcast([P, 2 * NSEL, NT]),
                op=mybir.AluOpType.mult)
            nc.vector.reduce_sum(red_sb[:, j, :], tmpS[:],
                                 axis=mybir.AxisListType.X)
            nc.vector.tensor_copy(redi_sb[:, j, :], red_sb[:, j, :NSEL])
        if dbg is not None:
            nc.sync.dma_start(
                dbg["dbg_red"][:].rearrange("p (j k) -> p j k", k=2 * NSEL),
                red_sb[:])

    # ---- load idx back, transpose to slot tiles, gather + transpose x ----
    with tc.tile_pool(name="gat_sb", bufs=2) as gpool, \
         tc.tile_pool(name="gat_ps", bufs=2, space="PSUM") as gps:
        idx_row = gpool.tile([1, C], f32, bufs=1, name="idx_row")
        nc.sync.dma_start(idx_row[:], idxr_dram[:].rearrange("c o -> o c"))
        pti = gps.tile([P, ST], f32, bufs=1, name="pti")
        for s in range(ST):
            nc.tensor.transpose(pti[:, s:s + 1],
                                idx_row[:, s * P:(s + 1) * P],
                                identf[:1, :1])
        idx_i = gpool.tile([P, ST], i32, bufs=1, name="idx_i")
        nc.vector.tensor_copy(idx_i[:], pti[:])
        if dbg is not None:
            nc.sync.dma_start(dbg["dbg_idx"][:], idx_i[:])
        for s in range(ST):
            xg = gpool.tile([P, D], bf16, tag="xg", name="xg")
            nc.gpsimd.indirect_dma_start(
                out=xg[:], out_offset=None, in_=xbf[:],
                in_offset=IndirectOffsetOnAxis(ap=idx_i[:, s:s + 1], axis=0),
            )
            for d in range(KD):
                pt = gps.tile([P, P], bf16, tag="pt", name="pt")
                nc.tensor.transpose(pt[:], xg[:, d * P:(d + 1) * P],
                                    ident[:])
                nc.vector.tensor_copy(xgT[d][:, s * P:(s + 1) * P], pt[:])
        if dbg is not None:
            nc.sync.dma_start(dbg["dbg_xgT"][:], xgT[0][:])

    # -------- FFN (bf16) + chunked y AllGather + overlapped combine --------
    with tc.tile_pool(name="ffn_sb", bufs=1, named_scope="ffn") as fpool, \
         tc.tile_pool(name="ffn_ps", bufs=1, space="PSUM") as fps, \
         tc.tile_pool(name="comb_sb", bufs=2, named_scope="combine") as cpool:
        for g in range(NG):
            TG = GS[g]
            t0 = GSTART[g] // P
            rows = GROWS[g]
            py = [[fps.tile([P, 512], f32, tag=f"py_{t}_{n}",
                            name=f"py_{t}_{n}")
                   for n in range(2)] for t in range(TG)]
            for f in range(NF):
                ph = fps.tile([P, CHK], f32, tag="ph", bufs=2, name="ph")
                for k in range(KD):
                    nc.tensor.matmul(
                        ph[:, :rows], lhsT=w1_sb[k][:, f * P:(f + 1) * P],
                        rhs=xgT[k][:, GSTART[g]:GSTART[g] + rows],
                        start=(k == 0), stop=(k == KD - 1))
                hbuf = fpool.tile([P, CHK], bf16, tag="hbuf", bufs=3,
                                  name="hbuf")
                nc.scalar.activation(hbuf[:, :rows], ph[:, :rows],
                                     mybir.ActivationFunctionType.Relu,
                                     bias=b1_sb[:, f:f + 1], scale=1.0)
                w2f = fpool.tile([P, D], bf16, tag="w2f", bufs=3, name="w2f")
                nc.sync.dma_start(w2f[:], w2d[f * P:(f + 1) * P, :])
                for t in range(TG):
                    for n in range(2):
                        nc.tensor.matmul(
                            py[t][n][:],
                            lhsT=hbuf[:, t * P:(t + 1) * P],
                            rhs=w2f[:, n * 512:(n + 1) * 512],
                            start=(f == 0), stop=False)
            # bias b2 via rank-1 matmul, then write out
            for t in range(TG):
                for n in range(2):
                    nc.tensor.matmul(
                        py[t][n][:], lhsT=ones_row[:],
                        rhs=b2_sb[:, n * 512:(n + 1) * 512],
                        start=False, stop=True)
                ysb = fpool.tile([P, D], bf16, tag="ysb", bufs=2, name="ysb")
                nc.vector.tensor_copy(ysb[:, :512], py[t][0][:])
                nc.vector.tensor_copy(ysb[:, 512:], py[t][1][:])
                nc.sync.dma_start(y_dram[g][t * P:(t + 1) * P, :], ysb[:])
            # ship this chunk while the next group computes
            nc.gpsimd.collective_compute(
                "AllGather", mybir.AluOpType.bypass,
                replica_groups=[list(range(N_CORES))],
                ins=[y_dram[g][:].opt()],
                outs=[y_all[g][:].opt()],
            )
            if dbg is not None:
                nc.sync.dma_start(dbg["dbg_y"][g][:], y_all[g][:])
            # combine this chunk's contributions while the next group runs
            for j in range(OWN_TILES):
                for q in range(2):
                    k = q * NG + g
                    yt = cpool.tile([P, D], bf16, tag=f"yt{g}{q}",
                                    name=f"yt{g}{q}")
                    nc.gpsimd.indirect_dma_start(
                        out=yt[:], out_offset=None, in_=y_all[g][:],
                        in_offset=IndirectOffsetOnAxis(
                            ap=redi_sb[:, j, k:k + 1], axis=0))
                    w = red_sb[:, j, NSEL + k:NSEL + k + 1]
                    if g == 0 and q == 0:
                        nc.vector.tensor_scalar(ot[j][:], yt[:], w, None,
                                                op0=mybir.AluOpType.mult)
                    else:
                        nc.vector.scalar_tensor_tensor(
                            out=ot[j][:], in0=yt[:], scalar=w, in1=ot[j][:],
                            op0=mybir.AluOpType.mult,
                            op1=mybir.AluOpType.add)
        for j in range(OWN_TILES):
            nc.sync.dma_start(out[j * P:(j + 1) * P, :], ot[j][:])

    persist.release()
    dram.release()


def _host_prep(x, Wg, bg, W1, b1, W2, b2, C):
    xf = np.ascontiguousarray(x.reshape(T, D).astype(np.float32))
    xT = np.ascontiguousarray(xf.T)
    xbf = xf.astype(ml_dtypes.bfloat16)
    bgb = np.tile(bg.astype(np.float32), NT)[None, :].repeat(P, 0)
    bgb = np.ascontiguousarray(bgb)
    l128 = np.triu(np.ones((P, P), np.float32), 1)  # [t', t] = 1 if t' < t
    in_maps = []
    for c in range(N_CORES):
        sel = np.zeros(E, np.float32)
        sel[c] = 1.0
        sel256 = np.ascontiguousarray(np.tile(sel, NT)[None, :].repeat(P, 0))
        ownm = np.zeros((P, OWN_TILES, NT), np.float32)
        for j in range(OWN_TILES):
            ownm[:, j, OWN_TILES * c + j] = 1.0
        in_maps.append({
            "xTs": np.ascontiguousarray(
                xT[:, c * TOK_PER_CORE:(c + 1) * TOK_PER_CORE]),
            "xbf": xbf,
            "w1d": np.ascontiguousarray(W1[c].astype(ml_dtypes.bfloat16)),
            "w2d": np.ascontiguousarray(W2[c].astype(ml_dtypes.bfloat16)),
            "wgd": np.ascontiguousarray(Wg.astype(np.float32)),
            "bgb": bgb,
            "b1pm": np.ascontiguousarray(
                b1[c].astype(np.float32).reshape(NF, P).T),
            "b2r": np.ascontiguousarray(b2[c].astype(np.float32)[None, :]),
            "sel256": sel256,
            "l128d": l128,
            "ownmd": np.ascontiguousarray(ownm.reshape(P, OWN_TILES * NT)),
        })
    return in_maps


def _capacity(x, Wg, bg):
    xf = x.reshape(T, D).astype(np.float32)
    logits = xf @ Wg.astype(np.float32) + bg.astype(np.float32)
    part = np.partition(logits, E - 2, axis=-1)
    m2 = part[:, E - 2:E - 1]
    counts = (logits >= m2).sum(0)
    return int(np.ceil((counts.max() + 16) / CHK) * CHK)


LAST_RESULT = None


def kernel(x, Wg, bg, W1, b1, W2, b2):
    global LAST_RESULT
    from concourse.bass_utils import run_bass_kernel_spmd

    x = np.asarray(x)
    C = _capacity(x, np.asarray(Wg), np.asarray(bg))
    debug = bool(os.environ.get("BASS_DEBUG_OUT"))
    key = (C, debug)
    if key not in _cache:
        _cache[key] = build_module(C, debug_out=debug)
    nc = _cache[key]
    in_maps = _host_prep(x, np.asarray(Wg), np.asarray(bg), np.asarray(W1),
                         np.asarray(b1), np.asarray(W2), np.asarray(b2), C)
    trace = bool(os.environ.get("BASS_TRACE"))
    if trace:
        _setup_axon_profile_hook()
    res = run_bass_kernel_spmd(nc, in_maps, core_ids=list(range(N_CORES)),
                               trace=trace)
    LAST_RESULT = res
    out = np.empty((T, D), np.float32)
    for c in range(N_CORES):
        out[c * TOK_PER_CORE:(c + 1) * TOK_PER_CORE] = res.results[c]["out"]
    return out.reshape(B, S, D)


def _setup_axon_profile_hook():
    """Provide antenv.axon_hooks (missing in this image) so trace=True works."""
    import types
    try:
        import antenv
        if "antenv.axon_hooks" not in sys.modules:
            hooks = types.ModuleType("antenv.axon_hooks")
            hooks._hook = None
            hooks.set_axon_ntff_profile_hook = \
                lambda h: setattr(hooks, "_hook", h)
            hooks.get_axon_ntff_profile_hook = lambda: hooks._hook
            sys.modules["antenv.axon_hooks"] = hooks
            antenv.axon_hooks = hooks
            from trn_agent_boot.trn_boot import _ntff_profile_via_ctypes
            hooks.set_axon_ntff_profile_hook(
                _ntff_profile_via_ctypes("/opt/axon/libaxon_pjrt.so"))
    except Exception as e:  # profiling is best-effort
        print(f"profile hook setup failed: {e}", file=sys.stderr)


# revision 17
# speedup vs baseline: 1.2443x; 1.2443x over previous
"""Expert-parallel top-2 MoE kernel for 8 Trainium2 NeuronCores.

Strategy (expert-parallel, sparse dispatch, per the sharding hint):
  - Router sharded over cores: core c computes fp32 logits for its 512-token
    shard on the TensorEngine (Wg stationary) in [E, 512] layout, AllGathers
    the contiguous [E,512] blocks, and transposes to token-partition layout
    on the TensorEngine; softmax/top-2 on-device (top-2 indicator built with
    5 full-width vector ops via masked second-max).
  - Core c owns expert c. Slot positions come from matmul-based exclusive
    cumsums (single full-width matmuls). Dispatch builds the slot->token map
    with ONE multi-column indirect-DMA scatter of token ids into a [C] DRAM
    vector (tokens not routed here get an out-of-bounds slot id and are
    skipped via bounds_check), loads it back contiguously, transposes it to
    slot-tile layout on the TensorEngine, then indirect-gathers the C routed
    token rows (one op per FFN group) and transposes them into xgT.
  - Two-layer FFN in bf16 over the routed tokens (capacity padded to a
    multiple of 384) in 3 groups of 384 slots; each group's unscaled outputs
    are AllGathered right after they finish, and the combine (one batched
    indirect gather + gate-weighted fp32 accumulation) runs while the next
    group's FFN computes. Only the last chunk's AllGather is exposed.
  - Each core returns its own 512-token shard; host concatenates.

Numerics: router fp32 (top-2 selection fidelity), FFN bf16 with fp32
accumulation in PSUM, combine in fp32.
"""

import os
import sys

import numpy as np

for _p in ("/opt/trn_rl_repo",):
    if _p not in sys.path:
        sys.path.append(_p)

import ml_dtypes

import concourse.bass as bass
import concourse.mybir as mybir
import concourse.tile as tile
from concourse import bacc
from concourse.bass import IndirectOffsetOnAxis
from concourse.masks import make_identity

# Problem shapes (fixed per spec)
B, S, D, E = 2, 2048, 1024, 8
T = B * S          # 4096 tokens
F = 4 * D          # 4096 ffn dim
P = 128            # partitions
NT = T // P        # 32 token tiles
KD = D // P        # 8 contraction tiles over D
NF = F // P        # 32 f tiles
TOK_PER_CORE = T // E   # 512
OWN_TILES = TOK_PER_CORE // P  # 4
N_CORES = E
TG = 3                       # slot tiles per FFN group
CHK = TG * P                 # 384: slot chunk for the chunked AllGather
BIGP = 100000.0              # OOB slot id for tokens not routed here

f32 = mybir.dt.float32
bf16 = mybir.dt.bfloat16
i32 = mybir.dt.int32
u32 = mybir.dt.uint32

_cache = {}


def build_module(C: int, debug_out: bool = False):
    """Build the SPMD Bass module for capacity C (multiple of 384)."""
    assert C % CHK == 0
    ST = C // P  # slot tiles per expert
    NG = ST // TG

    nc = bacc.Bacc("TRN2", target_bir_lowering=False, debug=False,
                   num_devices=N_CORES)

    # ---- I/O ----
    xTs = nc.dram_tensor("xTs", [D, TOK_PER_CORE], f32,
                         kind="ExternalInput").ap()
    xbf = nc.dram_tensor("xbf", [T, D], bf16, kind="ExternalInput").ap()
    w1d = nc.dram_tensor("w1d", [D, F], bf16, kind="ExternalInput").ap()
    w2d = nc.dram_tensor("w2d", [F, D], bf16, kind="ExternalInput").ap()
    wgd = nc.dram_tensor("wgd", [D, E], f32, kind="ExternalInput").ap()
    bgb = nc.dram_tensor("bgb", [P, NT * E], f32, kind="ExternalInput").ap()
    b1pm = nc.dram_tensor("b1pm", [P, NF], f32, kind="ExternalInput").ap()
    b2r = nc.dram_tensor("b2r", [1, D], f32, kind="ExternalInput").ap()
    sel256 = nc.dram_tensor("sel256", [P, NT * E], f32,
                            kind="ExternalInput").ap()
    l128d = nc.dram_tensor("l128d", [P, P], f32, kind="ExternalInput").ap()
    ownmd = nc.dram_tensor("ownmd", [P, OWN_TILES * NT], f32,
                           kind="ExternalInput").ap()
    out = nc.dram_tensor("out", [TOK_PER_CORE, D], f32,
                         kind="ExternalOutput").ap()
    dbg = None
    if debug_out:
        dbg = {
            "dbg_l": nc.dram_tensor("dbg_l", [P, NT * E], f32,
                                    kind="ExternalOutput").ap(),
            "dbg_pos": nc.dram_tensor("dbg_pos", [P, NT * E], f32,
                                      kind="ExternalOutput").ap(),
            "dbg_ind": nc.dram_tensor("dbg_ind", [P, NT * E], f32,
                                      kind="ExternalOutput").ap(),
            "dbg_posm": nc.dram_tensor("dbg_posm", [P, NT], f32,
                                       kind="ExternalOutput").ap(),
            "dbg_idx": nc.dram_tensor("dbg_idx", [P, ST], i32,
                                      kind="ExternalOutput").ap(),
            "dbg_xgT": nc.dram_tensor("dbg_xgT", [P, C], bf16,
                                      kind="ExternalOutput").ap(),
            "dbg_red": nc.dram_tensor(
                "dbg_red", [P, OWN_TILES * 4 * NG], f32,
                kind="ExternalOutput").ap(),
            "dbg_y": [nc.dram_tensor(f"dbg_y{g}", [N_CORES * CHK, D],
                                     bf16, kind="ExternalOutput").ap()
                      for g in range(NG)],
        }

    with tile.TileContext(nc) as tc:
        _emit(tc, C, ST, xTs, xbf, w1d, w2d, wgd, bgb, b1pm, b2r, sel256,
              l128d, ownmd, out, dbg)

    nc.compile()
    return nc


def _emit(tc, C, ST, xTs, xbf, w1d, w2d, wgd, bgb, b1pm, b2r, sel256,
          l128d, ownmd, out, dbg=None):
    nc = tc.nc
    NE = NT * E  # 256
    NG = ST // TG
    NSEL = 2 * NG

    # ---------------- persistent pools ----------------
    persist = tc.alloc_tile_pool(name="persist", bufs=1)
    dram = tc.alloc_tile_pool(name="dram", bufs=1, space="DRAM")

    # tiny warmup AllGather: absorbs first-collective setup cost and aligns
    # the cores before the logits AllGather on the critical path
    wup_in = dram.tile([E, 4], f32, name="wup_in")
    wup_out = dram.tile([N_CORES * E, 4], f32, addr_space="Shared",
                        name="wup_out")
    nc.gpsimd.collective_compute(
        "AllGather", mybir.AluOpType.bypass,
        replica_groups=[list(range(N_CORES))],
        ins=[wup_in[:].opt()], outs=[wup_out[:].opt()],
    )

    # router inputs first (split across DMA queues so the router starts fast)
    wg_sb = persist.tile([P, KD, E], f32, name="wg_sb")
    nc.sync.dma_start(wg_sb[:], wgd.rearrange("(k p) e -> p k e", p=P))
    xs = persist.tile([P, KD, TOK_PER_CORE], f32, name="xs")
    xsv = xTs.rearrange("(k p) t -> p k t", p=P)
    for k in range(KD):
        nc.sync.dma_start(xs[:, k, :], xsv[:, k, :])
    bg_sb = persist.tile([P, NE], f32, name="bg_sb")
    nc.sync.dma_start(bg_sb[:], bgb[:])
    sel_sb = persist.tile([P, NE], f32, name="sel_sb")
    nc.sync.dma_start(sel_sb[:], sel256[:])
    l128_sb = persist.tile([P, P], f32, name="l128_sb")
    nc.sync.dma_start(l128_sb[:], l128d[:])
    ownm_sb = persist.tile([P, OWN_TILES * NT], f32, name="ownm_sb")
    nc.sync.dma_start(ownm_sb[:], ownmd[:])
    b1_sb = persist.tile([P, NF], f32, name="b1_sb")
    nc.sync.dma_start(b1_sb[:], b1pm[:])
    b2_sb = persist.tile([1, D], f32, name="b2_sb")
    nc.sync.dma_start(b2_sb[:], b2r[:])
    ident = persist.tile([P, P], bf16, name="ident")
    make_identity(nc, ident[:])
    identf = persist.tile([P, P], f32, name="identf")
    make_identity(nc, identf[:])
    ones_col = persist.tile([P, 1], f32, name="ones_col")
    nc.vector.memset(ones_col[:], 1.0)
    ones_row = persist.tile([1, P], f32, name="ones_row")
    nc.vector.memset(ones_row[:], 1.0)
    # slot iota (f32) and the [p, tile] weight pair for the idx-build matmuls
    iotaC = persist.tile([P, C], f32, name="iotaC")
    iotaC_i = persist.tile([P, C], i32, name="iotaC_i")
    nc.gpsimd.iota(iotaC_i[:], pattern=[[1, C]], base=0, channel_multiplier=0)
    nc.vector.tensor_copy(iotaC[:], iotaC_i[:])
    pv2 = persist.tile([P, NT, 2], bf16, name="pv2")
    pv2_i = persist.tile([P, NT, 2], i32, name="pv2_i")
    nc.gpsimd.iota(pv2_i[:, :, 0], pattern=[[0, NT]], base=0,
                   channel_multiplier=1)
    nc.gpsimd.iota(pv2_i[:, :, 1], pattern=[[1, NT]], base=0,
                   channel_multiplier=0)
    nc.vector.tensor_copy(pv2[:], pv2_i[:])

    # router / dispatch state kept for the combine phase
    exp_all = persist.tile([P, NE], f32, name="exp_all")    # exp(logits)
    m8_all = persist.tile([P, NE], f32, name="m8_all")      # per-tile top8
    r_all = persist.tile([P, NT], f32, name="r_all")        # 1/sum(exp)
    pos_all = persist.tile([P, NE], f32, name="pos_all")    # excl cumsum
    ind_all = persist.tile([P, NE], f32, name="ind_all")    # top2 indicator
    ei_all = persist.tile([P, NE], u32, name="ei_all")      # top8 indices
    # selection data, plane index k = 2*g + q (group-major for batched gather)
    red_sb = persist.tile([P, OWN_TILES, 2 * NSEL], f32, name="red_sb")
    redi_sb = persist.tile([P, OWN_TILES, NSEL], i32, name="redi_sb")
    ot = [persist.tile([P, D], f32, name=f"ot{j}") for j in range(OWN_TILES)]

    xgT = [persist.tile([P, C], bf16, name=f"xgT{d}") for d in range(KD)]

    l_dram = dram.tile([E, TOK_PER_CORE], f32, name="l_dram")
    lg_dram = dram.tile([N_CORES * E, TOK_PER_CORE], f32, addr_space="Shared",
                        name="lg_dram")
    y_dram = [dram.tile([CHK, D], bf16, name=f"y_dram{g}") for g in range(NG)]
    y_all = [dram.tile([N_CORES * CHK, D], bf16, addr_space="Shared",
                       name=f"y_all{g}") for g in range(NG)]

    # ---------------- router (sharded + AllGather) ----------------
    with tc.tile_pool(name="router_sb", bufs=1, named_scope="router") as rpool, \
         tc.tile_pool(name="router_ps", bufs=1, space="PSUM") as rps:
        lT = rps.tile([E, TOK_PER_CORE], f32, name="lT")
        for k in range(KD):
            nc.tensor.matmul(lT[:], lhsT=wg_sb[:, k, :], rhs=xs[:, k, :],
                             start=(k == 0), stop=(k == KD - 1))
        lt_sb = rpool.tile([E, TOK_PER_CORE], f32, name="lt_sb")
        nc.vector.tensor_copy(lt_sb[:], lT[:])
        nc.sync.dma_start(l_dram[:], lt_sb[:])
        nc.gpsimd.collective_compute(
            "AllGather", mybir.AluOpType.bypass,
            replica_groups=[list(range(N_CORES))],
            ins=[l_dram[:].opt()], outs=[lg_dram[:].opt()],
        )
        # stream gathered logits per core (sources at partition 0), then
        # transpose each 128-token tile into one PSUM tile; single copy out
        l_all = rpool.tile([P, NE], f32, name="l_all")
        QT = TOK_PER_CORE // P  # 4 token tiles per core shard
        pt_all = rps.tile([P, N_CORES, QT, E], f32, name="pt_all")
        for c in range(N_CORES):
            lgc = rpool.tile([E, TOK_PER_CORE], f32, tag="lgc", bufs=2,
                             name="lgc")
            nc.sync.dma_start(lgc[:], lg_dram[c * E:(c + 1) * E, :])
            for q in range(QT):
                nc.tensor.transpose(
                    pt_all[:, c, q, :], lgc[:, q * P:(q + 1) * P],
                    identf[:E, :E])
        nc.vector.tensor_copy(l_all[:], pt_all[:])
        nc.vector.tensor_add(l_all[:], l_all[:], bg_sb[:])
        if dbg is not None:
            nc.sync.dma_start(dbg["dbg_l"][:], l_all[:])
        nc.scalar.activation(exp_all[:], l_all[:],
                             mybir.ActivationFunctionType.Exp)
        # top-2 indicator via masked second-max (5 full-width ops)
        exp3 = exp_all[:].rearrange("p (t e) -> p t e", e=E)
        mx = rpool.tile([P, NT], f32, name="mx")
        nc.vector.reduce_max(mx[:], exp3, axis=mybir.AxisListType.X)
        eqm = rpool.tile([P, NE], f32, name="eqm")
        nc.vector.tensor_tensor(
            out=eqm[:].rearrange("p (t e) -> p t e", e=E), in0=exp3,
            in1=mx[:, :, None].to_broadcast([P, NT, E]),
            op=mybir.AluOpType.is_ge)
        nm = rpool.tile([P, NE], f32, name="nm")
        nc.vector.scalar_tensor_tensor(
            out=nm[:], in0=eqm[:], scalar=-BIGP, in1=exp_all[:],
            op0=mybir.AluOpType.mult, op1=mybir.AluOpType.add)
        m2 = rpool.tile([P, NT], f32, name="m2")
        nc.vector.reduce_max(m2[:], nm[:].rearrange("p (t e) -> p t e", e=E),
                             axis=mybir.AxisListType.X)
        nc.vector.tensor_tensor(
            out=ind_all[:].rearrange("p (t e) -> p t e", e=E), in0=exp3,
            in1=m2[:, :, None].to_broadcast([P, NT, E]),
            op=mybir.AluOpType.is_ge)

    # ---------------- dispatch: cumsum positions + id scatter ----------------
    with tc.tile_pool(name="disp_sb", bufs=1, named_scope="dispatch") as dpool, \
         tc.tile_pool(name="disp_ps", bufs=1, space="PSUM") as dps:
        # per-(tile,expert) totals in one matmul
        ptot = dps.tile([1, NE], f32, name="ptot")
        nc.tensor.matmul(ptot[:], lhsT=ones_col[:], rhs=ind_all[:],
                         start=True, stop=True)
        tot_flat = dpool.tile([1, NE], f32, name="tot_flat")
        nc.vector.tensor_copy(tot_flat[:], ptot[:])
        # reshape [1, NT*E] -> [NT, E] via sbuf-to-sbuf DMA
        tot32 = dpool.tile([NT, E], f32, name="tot32")
        nc.sync.dma_start(tot32[:], tot_flat[:])
        # exclusive cumsum over tiles: strict-lower matmul
        pofs = dps.tile([NT, E], f32, name="pofs")
        nc.tensor.matmul(pofs[:], lhsT=l128_sb[:NT, :NT], rhs=tot32[:],
                         start=True, stop=True)
        ofs32 = dpool.tile([NT, E], f32, name="ofs32")
        nc.vector.tensor_copy(ofs32[:], pofs[:])
        ofs_flat = dpool.tile([1, NE], f32, name="ofs_flat")
        nc.sync.dma_start(ofs_flat[:], ofs32[:])
        # positions: local excl cumsum + broadcast tile offset, one pass
        ppos = dps.tile([P, NE], f32, name="ppos")
        nc.tensor.matmul(ppos[:], lhsT=l128_sb[:], rhs=ind_all[:],
                         start=True, stop=False)
        nc.tensor.matmul(ppos[:], lhsT=ones_row[:], rhs=ofs_flat[:],
                         start=False, stop=True)
        nc.vector.tensor_copy(pos_all[:], ppos[:])

        # my expert's masked positions: ind ? pos : BIGP (skipped as OOB)
        tmp = dpool.tile([P, NE], f32, name="tmp")
        nc.vector.tensor_scalar_add(tmp[:], pos_all[:], -BIGP)
        nc.vector.tensor_mul(tmp[:], tmp[:], ind_all[:])
        nc.vector.tensor_mul(tmp[:], tmp[:], sel_sb[:])
        pos_m = dpool.tile([P, NT], f32, name="pos_m")
        nc.vector.reduce_sum(pos_m[:], tmp[:].rearrange(
            "p (t e) -> p t e", e=E), axis=mybir.AxisListType.X)
        nc.vector.tensor_scalar_add(pos_m[:], pos_m[:], BIGP)
        if dbg is not None:
            nc.sync.dma_start(dbg["dbg_pos"][:], pos_all[:])
            nc.sync.dma_start(dbg["dbg_ind"][:], ind_all[:])
            nc.sync.dma_start(dbg["dbg_posm"][:], pos_m[:])

        # slot->token map via one-hot matmuls: Pt[p, s] = (pos_m[p,t]==s);
        # acc[0, s] = token partition p at slot s, acc[1, s] = its tile id
        acc = [dps.tile([2, CHK], f32, tag=f"accx{ch}", name=f"accx{ch}")
               for ch in range(C // CHK)]
        for tt in range(NT):
            Pt = dpool.tile([P, C], bf16, tag="Pt", bufs=4, name="Pt")
            nc.vector.tensor_scalar(
                Pt[:], iotaC[:], pos_m[:, tt:tt + 1], None,
                op0=mybir.AluOpType.is_equal)
            for ch in range(C // CHK):
                nc.tensor.matmul(acc[ch][:], lhsT=pv2[:, tt, :],
                                 rhs=Pt[:, ch * CHK:(ch + 1) * CHK],
                                 start=(tt == 0), stop=(tt == NT - 1))
        idx2_sb = dpool.tile([2, C], f32, name="idx2_sb")
        for ch in range(C // CHK):
            nc.vector.tensor_copy(idx2_sb[:, ch * CHK:(ch + 1) * CHK],
                                  acc[ch][:])

    # expert weights W1: issued after the router/dispatch DMAs so startup
    # queue contention doesn't delay the logits AllGather
    w1_sb = [persist.tile([P, F], bf16, name=f"w1_sb{k}") for k in range(KD)]
    for k in range(KD):
        nc.sync.dma_start(w1_sb[k][:], w1d[k * P:(k + 1) * P, :])

    # ---- finish idx (transpose + fold tile id), gather + transpose x ----
    with tc.tile_pool(name="gat_sb", bufs=2) as gpool, \
         tc.tile_pool(name="gat_ps", bufs=2, space="PSUM") as gps:
        pti = gps.tile([P, ST, 2], f32, bufs=1, name="pti")
        for s in range(ST):
            nc.tensor.transpose(pti[:, s, :], idx2_sb[:, s * P:(s + 1) * P],
                                identf[:2, :2])
        pti_sb = gpool.tile([P, ST, 2], f32, bufs=1, name="pti_sb")
        nc.vector.tensor_copy(pti_sb[:], pti[:])
        idx_f = gpool.tile([P, ST], f32, bufs=1, name="idx_f")
        nc.vector.scalar_tensor_tensor(
            out=idx_f[:], in0=pti_sb[:, :, 1], scalar=float(P),
            in1=pti_sb[:, :, 0], op0=mybir.AluOpType.mult,
            op1=mybir.AluOpType.add)
        idx_i = gpool.tile([P, ST], i32, bufs=1, name="idx_i")
        nc.vector.tensor_copy(idx_i[:], idx_f[:])
        if dbg is not None:
            nc.sync.dma_start(dbg["dbg_idx"][:], idx_i[:])
        for s in range(ST):
            xg = gpool.tile([P, D], bf16, tag="xg", bufs=3, name="xg")
            nc.gpsimd.indirect_dma_start(
                out=xg[:], out_offset=None, in_=xbf[:],
                in_offset=IndirectOffsetOnAxis(ap=idx_i[:, s:s + 1], axis=0),
            )
            for d in range(KD):
                pt = gps.tile([P, P], bf16, tag="pt", name="pt")
                nc.tensor.transpose(pt[:], xg[:, d * P:(d + 1) * P],
                                    ident[:])
                nc.vector.tensor_copy(xgT[d][:, s * P:(s + 1) * P], pt[:])
        if dbg is not None:
            nc.sync.dma_start(dbg["dbg_xgT"][:], xgT[0][:])

    with tc.tile_pool(name="sel_sb", bufs=1, named_scope="select") as spool:
        # off the critical path: top-8 values/indices + softmax denom + the
        # combine selection stack (vector work that overlaps the scatter)
        for tt in range(NT):
            sl = slice(tt * E, (tt + 1) * E)
            nc.vector.max(out=m8_all[:, sl], in_=exp_all[:, sl])
            nc.vector.max_index(out=ei_all[:, sl], in_max=m8_all[:, sl],
                                in_values=exp_all[:, sl])
        s_all = spool.tile([P, NT], f32, name="s_all")
        nc.vector.reduce_sum(s_all[:], exp_all[:].rearrange(
            "p (t e) -> p t e", e=E), axis=mybir.AxisListType.X)
        nc.vector.reciprocal(r_all[:], s_all[:])

        # selection stack: NSEL offset planes + NSEL weight planes [P, NT],
        # plane k = 2*g + q (group-major so per-group gathers batch)
        e1f = spool.tile([P, NT], f32, name="e1f")
        e2f = spool.tile([P, NT], f32, name="e2f")
        ei3 = ei_all[:].rearrange("p (t e) -> p t e", e=E)
        nc.vector.tensor_copy(e1f[:], ei3[:, :, 0])
        nc.vector.tensor_copy(e2f[:], ei3[:, :, 1])
        ioz = spool.tile([P, NE], i32, name="ioz")
        nc.gpsimd.iota(ioz[:].rearrange("p (t e) -> p t e", e=E),
                       pattern=[[0, NT], [1, E]], base=0, channel_multiplier=0)
        iof = spool.tile([P, NE], f32, name="iof")
        nc.vector.tensor_copy(iof[:], ioz[:])
        m83 = m8_all[:].rearrange("p (t e) -> p t e", e=E)
        Ssel = spool.tile([P, 2 * NSEL, NT], f32, name="Ssel")
        for q, ef in ((0, e1f), (1, e2f)):
            oh = spool.tile([P, NE], f32, tag=f"oh{q}", name=f"oh{q}")
            nc.vector.tensor_tensor(
                out=oh[:].rearrange("p (t e) -> p t e", e=E),
                in0=iof[:].rearrange("p (t e) -> p t e", e=E),
                in1=ef[:, :, None].to_broadcast([P, NT, E]),
                op=mybir.AluOpType.is_equal)
            nc.vector.tensor_mul(oh[:], oh[:], pos_all[:])
            slot = spool.tile([P, NT], f32, tag=f"slot{q}", name=f"slot{q}")
            nc.vector.reduce_sum(slot[:], oh[:].rearrange(
                "p (t e) -> p t e", e=E), axis=mybir.AxisListType.X)
            gch = spool.tile([P, NT], f32, tag=f"gch{q}", name=f"gch{q}")
            nc.vector.tensor_scalar(gch[:], slot[:], float(CHK), None,
                                    op0=mybir.AluOpType.is_ge)
            for gg in range(2, NG):
                t2 = spool.tile([P, NT], f32, tag="t2", name="t2")
                nc.vector.tensor_scalar(t2[:], slot[:], float(CHK * gg), None,
                                        op0=mybir.AluOpType.is_ge)
                nc.vector.tensor_add(gch[:], gch[:], t2[:])
            base = spool.tile([P, NT], f32, tag=f"base{q}", name=f"base{q}")
            nc.vector.scalar_tensor_tensor(
                out=base[:], in0=ef[:], scalar=float(CHK), in1=slot[:],
                op0=mybir.AluOpType.mult, op1=mybir.AluOpType.add)
            gv = spool.tile([P, NT], f32, tag=f"gv{q}", name=f"gv{q}")
            nc.vector.tensor_tensor(out=gv[:], in0=m83[:, :, q], in1=r_all[:],
                                    op=mybir.AluOpType.mult)
            for gg in range(NG):
                k = 2 * gg + q
                eq = spool.tile([P, NT], f32, tag="eq", name="eq")
                nc.vector.tensor_scalar(eq[:], gch[:], float(gg), None,
                                        op0=mybir.AluOpType.is_equal)
                nc.vector.tensor_scalar_add(Ssel[:, k, :], base[:],
                                            float(-CHK * gg))
                nc.vector.tensor_mul(Ssel[:, k, :], Ssel[:, k, :], eq[:])
                nc.vector.tensor_mul(Ssel[:, NSEL + k, :], eq[:], gv[:])
        for j in range(OWN_TILES):
            own = ownm_sb[:, j * NT:(j + 1) * NT]
            tmpS = spool.tile([P, 2 * NSEL, NT], f32, tag="tmpS", bufs=2,
                              name="tmpS")
            nc.vector.tensor_tensor(
                out=tmpS[:], in0=Ssel[:],
                in1=own[:, None, :].to_broadcast([P, 2 * NSEL, NT]),
                op=mybir.AluOpType.mult)
            nc.vector.reduce_sum(red_sb[:, j, :], tmpS[:],
                                 axis=mybir.AxisListType.X)
            nc.vector.tensor_copy(redi_sb[:, j, :], red_sb[:, j, :NSEL])
        if dbg is not None:
            nc.sync.dma_start(
                dbg["dbg_red"][:].rearrange("p (j k) -> p j k", k=2 * NSEL),
                red_sb[:])


    # -------- FFN (bf16) + chunked y AllGather + overlapped combine --------
    with tc.tile_pool(name="ffn_sb", bufs=1, named_scope="ffn") as fpool, \
         tc.tile_pool(name="ffn_ps", bufs=1, space="PSUM") as fps, \
         tc.tile_pool(name="comb_sb", bufs=2, named_scope="combine") as cpool:
        for g in range(NG):
            t0 = g * TG
            py = [[fps.tile([P, 512], f32, tag=f"py_{t}_{n}",
                            name=f"py_{t}_{n}")
                   for n in range(2)] for t in range(TG)]
            for f in range(NF):
                ph = fps.tile([P, CHK], f32, tag="ph", bufs=2, name="ph")
                for k in range(KD):
                    nc.tensor.matmul(
                        ph[:], lhsT=w1_sb[k][:, f * P:(f + 1) * P],
                        rhs=xgT[k][:, t0 * P:t0 * P + CHK],
                        start=(k == 0), stop=(k == KD - 1))
                hbuf = fpool.tile([P, CHK], bf16, tag="hbuf", bufs=3,
                                  name="hbuf")
                nc.scalar.activation(hbuf[:], ph[:],
                                     mybir.ActivationFunctionType.Relu,
                                     bias=b1_sb[:, f:f + 1], scale=1.0)
                w2f = fpool.tile([P, D], bf16, tag="w2f", bufs=3, name="w2f")
                nc.sync.dma_start(w2f[:], w2d[f * P:(f + 1) * P, :])
                for t in range(TG):
                    for n in range(2):
                        nc.tensor.matmul(
                            py[t][n][:],
                            lhsT=hbuf[:, t * P:(t + 1) * P],
                            rhs=w2f[:, n * 512:(n + 1) * 512],
                            start=(f == 0), stop=False)
            # bias b2 via rank-1 matmul, then write out
            for t in range(TG):
                for n in range(2):
                    nc.tensor.matmul(
                        py[t][n][:], lhsT=ones_row[:],
                        rhs=b2_sb[:, n * 512:(n + 1) * 512],
                        start=False, stop=True)
                ysb = fpool.tile([P, D], bf16, tag="ysb", bufs=2, name="ysb")
                nc.vector.tensor_copy(ysb[:, :512], py[t][0][:])
                nc.vector.tensor_copy(ysb[:, 512:], py[t][1][:])
                nc.sync.dma_start(y_dram[g][t * P:(t + 1) * P, :], ysb[:])
            # ship this chunk while the next group computes
            nc.gpsimd.collective_compute(
                "AllGather", mybir.AluOpType.bypass,
                replica_groups=[list(range(N_CORES))],
                ins=[y_dram[g][:].opt()],
                outs=[y_all[g][:].opt()],
            )
            if dbg is not None:
                nc.sync.dma_start(dbg["dbg_y"][g][:], y_all[g][:])
            # combine this chunk: one batched gather of all (j, q) rows,
            # then gate-weighted accumulation while the next group runs
            yt = cpool.tile([P, OWN_TILES * 2, D], bf16, tag=f"yt{g}",
                            bufs=1, name=f"yt{g}")
            for j in range(OWN_TILES):
                for q in range(2):
                    nc.gpsimd.indirect_dma_start(
                        out=yt[:, 2 * j + q, :], out_offset=None,
                        in_=y_all[g][:],
                        in_offset=IndirectOffsetOnAxis(
                            ap=redi_sb[:, j, 2 * g + q:2 * g + q + 1],
                            axis=0))
            for j in range(OWN_TILES):
                for q in range(2):
                    k = 2 * g + q
                    w = red_sb[:, j, NSEL + k:NSEL + k + 1]
                    if g == 0 and q == 0:
                        nc.vector.tensor_scalar(ot[j][:], yt[:, 2 * j + q, :],
                                                w, None,
                                                op0=mybir.AluOpType.mult)
                    else:
                        nc.vector.scalar_tensor_tensor(
                            out=ot[j][:], in0=yt[:, 2 * j + q, :], scalar=w,
                            in1=ot[j][:], op0=mybir.AluOpType.mult,
                            op1=mybir.AluOpType.add)
        for j in range(OWN_TILES):
            nc.sync.dma_start(out[j * P:(j + 1) * P, :], ot[j][:])

    persist.release()
    dram.release()


def _host_prep(x, Wg, bg, W1, b1, W2, b2, C):
    xf = np.ascontiguousarray(x.reshape(T, D).astype(np.float32))
    xT = np.ascontiguousarray(xf.T)
    xbf = xf.astype(ml_dtypes.bfloat16)
    bgb = np.tile(bg.astype(np.float32), NT)[None, :].repeat(P, 0)
    bgb = np.ascontiguousarray(bgb)
    l128 = np.triu(np.ones((P, P), np.float32), 1)  # [t', t] = 1 if t' < t
    in_maps = []
    for c in range(N_CORES):
        sel = np.zeros(E, np.float32)
        sel[c] = 1.0
        sel256 = np.ascontiguousarray(np.tile(sel, NT)[None, :].repeat(P, 0))
        ownm = np.zeros((P, OWN_TILES, NT), np.float32)
        for j in range(OWN_TILES):
            ownm[:, j, OWN_TILES * c + j] = 1.0
        in_maps.append({
            "xTs": np.ascontiguousarray(
                xT[:, c * TOK_PER_CORE:(c + 1) * TOK_PER_CORE]),
            "xbf": xbf,
            "w1d": np.ascontiguousarray(W1[c].astype(ml_dtypes.bfloat16)),
            "w2d": np.ascontiguousarray(W2[c].astype(ml_dtypes.bfloat16)),
            "wgd": np.ascontiguousarray(Wg.astype(np.float32)),
            "bgb": bgb,
            "b1pm": np.ascontiguousarray(
                b1[c].astype(np.float32).reshape(NF, P).T),
            "b2r": np.ascontiguousarray(b2[c].astype(np.float32)[None, :]),
            "sel256": sel256,
            "l128d": l128,
            "ownmd": np.ascontiguousarray(ownm.reshape(P, OWN_TILES * NT)),
        })
    return in_maps


def _capacity(x, Wg, bg):
    xf = x.reshape(T, D).astype(np.float32)
    logits = xf @ Wg.astype(np.float32) + bg.astype(np.float32)
    part = np.partition(logits, E - 2, axis=-1)
    m2 = part[:, E - 2:E - 1]
    counts = (logits >= m2).sum(0)
    return int(np.ceil((counts.max() + 16) / CHK) * CHK)


LAST_RESULT = None


def kernel(x, Wg, bg, W1, b1, W2, b2):
    global LAST_RESULT
    from concourse.bass_utils import run_bass_kernel_spmd

    x = np.asarray(x)
    C = _capacity(x, np.asarray(Wg), np.asarray(bg))
    debug = bool(os.environ.get("BASS_DEBUG_OUT"))
    key = (C, debug)
    if key not in _cache:
        _cache[key] = build_module(C, debug_out=debug)
    nc = _cache[key]
    in_maps = _host_prep(x, np.asarray(Wg), np.asarray(bg), np.asarray(W1),
                         np.asarray(b1), np.asarray(W2), np.asarray(b2), C)
    trace = bool(os.environ.get("BASS_TRACE"))
    if trace:
        _setup_axon_profile_hook()
    res = run_bass_kernel_spmd(nc, in_maps, core_ids=list(range(N_CORES)),
                               trace=trace)
    LAST_RESULT = res
    out = np.empty((T, D), np.float32)
    for c in range(N_CORES):
        out[c * TOK_PER_CORE:(c + 1) * TOK_PER_CORE] = res.results[c]["out"]
    return out.reshape(B, S, D)


def _setup_axon_profile_hook():
    """Provide antenv.axon_hooks (missing in this image) so trace=True works."""
    import types
    try:
        import antenv
        if "antenv.axon_hooks" not in sys.modules:
            hooks = types.ModuleType("antenv.axon_hooks")
            hooks._hook = None
            hooks.set_axon_ntff_profile_hook = \
                lambda h: setattr(hooks, "_hook", h)
            hooks.get_axon_ntff_profile_hook = lambda: hooks._hook
            sys.modules["antenv.axon_hooks"] = hooks
            antenv.axon_hooks = hooks
            from trn_agent_boot.trn_boot import _ntff_profile_via_ctypes
            hooks.set_axon_ntff_profile_hook(
                _ntff_profile_via_ctypes("/opt/axon/libaxon_pjrt.so"))
    except Exception as e:  # profiling is best-effort
        print(f"profile hook setup failed: {e}", file=sys.stderr)
